# revision 8
# baseline (speedup 1.0000x reference)
"""Mode-adaptive linear (MoE soft routing) Trainium2 kernel.

out[b, o] = sum_c weights[b, c] * (inputs[b, :] @ w[c])[o] + (weights @ bias)[b, o]

Strategy: data-parallel shard of the batch across 8 NeuronCores (1024 rows
each); w/bias replicated.  Host stages the inputs in matmul-ready form:
x pre-transposed to [D_IN, B_SHARD] bf16, w as bf16, and the routing weights
replicated across partitions (wb[p, c, b] = weights[b, c]).  On-device the
routing weights are folded into the x^T tiles with one DVE multiply per
(expert-pair, batch-tile), and all 8 expert matmuls plus the bias matmul
accumulate into a single PSUM bank per 128-row batch tile (8 tiles = 8 banks,
each a K=4104 accumulation).  Experts stream from HBM in 4 pairs so the PE
never waits on the bulk weight DMA.
"""

import json
import types

import ml_dtypes
import numpy as np

import concourse.bass as bass
import concourse.mybir as mybir
import concourse.tile as tile
from concourse.bass import ts
from concourse.bass_utils import run_bass_kernel_spmd

N_CORES = 8
B, D_IN, D_OUT, N_CTRL = 8192, 512, 512, 8
B_SHARD = B // N_CORES          # 1024 rows per core
P = 128
N_TILES = B_SHARD // P          # 8 batch tiles per core
KS = D_IN // P                  # 4 K-chunks of 128
GROUPS = 4                      # expert pairs
CPG = N_CTRL // GROUPS          # 2 experts per group
F32 = mybir.dt.float32
BF16 = mybir.dt.bfloat16
N_WARM = 6


def _body(nc: bass.Bass, tc: tile.TileContext, xt_d, wb_d, wttb_d, w_d, o_d):
    with (
        tc.tile_pool(name="const", bufs=1) as const_pool,
        tc.tile_pool(name="xspool", bufs=10) as xspool,
        tc.tile_pool(name="opool", bufs=3) as opool,
        tc.tile_pool(name="mm_ps", bufs=8, space="PSUM") as mm_psum,
    ):
        # --- Warmup: PE p-state / HAM ramps to full clock only after a few
        # us of continuous activity.  A handful of dummy matmuls on a memset
        # tile bridge the gap until the first real (bias) matmuls; the 8 bias
        # matmuls then continue the ramp with real work.  The warm psum tile
        # shares bank 0 with batch-tile 7's accumulator (PE is in-order, so
        # the warmups retire long before tile 7's group starts).
        warm_sb = const_pool.tile([P, 512], BF16)
        nc.gpsimd.memset(warm_sb, 0.0)
        warm_ps = mm_psum.tile([P, 512], F32, tag="acc")
        for _ in range(N_WARM):
            nc.tensor.matmul(
                warm_ps, lhsT=warm_sb[:, 0:P], rhs=warm_sb,
                start=True, stop=True,
            )

        # --- Input DMAs, spread across four engine queues so the configs
        # (~600ns each) dispatch in parallel, ordered so the operands of the
        # first expert-pair land first.
        # scalar: routing weights + bias (tiny, gates the bias matmuls).
        wttb = const_pool.tile([N_CTRL, B_SHARD + D_OUT], BF16)
        nc.scalar.dma_start(wttb, wttb_d)

        # scalar also carries x^T (leading batch-tile pair first) + the first
        # expert-pair's wb.
        xt_sb = const_pool.tile([P, KS, B_SHARD], BF16)
        xt_src = xt_d.rearrange("(k p) b -> p k b", p=P)
        wb = const_pool.tile([P, N_CTRL, B_SHARD], BF16)
        nc.scalar.dma_start(xt_sb[:, :, 0:256], xt_src[:, :, 0:256])
        nc.scalar.dma_start(wb[:, 0:CPG], wb_d[:, 0:CPG])

        # gpsimd (software DGE, own queue): the rest of x^T and wb.
        nc.gpsimd.dma_start(xt_sb[:, :, 256:B_SHARD], xt_src[:, :, 256:B_SHARD])
        nc.gpsimd.dma_start(wb[:, CPG:N_CTRL], wb_d[:, CPG:N_CTRL])

        # sync: expert weights, finely chunked at the leading edge so the
        # first matmuls never wait on the bulk.
        w_sb = const_pool.tile([P, N_CTRL, KS, D_OUT], BF16)
        w_src = w_d.rearrange("c (k p) o -> p c k o", p=P)
        nc.sync.dma_start(w_sb[:, 0, 0], w_src[:, 0, 0])
        nc.sync.dma_start(w_sb[:, 0, 1:KS], w_src[:, 0, 1:KS])
        nc.sync.dma_start(w_sb[:, 1], w_src[:, 1])
        nc.sync.dma_start(w_sb[:, 2:4], w_src[:, 2:4])
        nc.sync.dma_start(w_sb[:, 4:6], w_src[:, 4:6])
        nc.sync.dma_start(w_sb[:, 6:8], w_src[:, 6:8])

        # --- Bias matmuls first: they only need the tiny wttb DMA, so they
        # start earliest, open all 8 psum accumulation groups, and double as
        # p-state ramp work.  lhsT = weights^T tile (K=8), rhs = bias.
        ps = [None] * N_TILES
        b_sb = wttb[:, B_SHARD:]
        for t in range(N_TILES):
            ps[t] = mm_psum.tile([P, D_OUT], F32, tag="acc", name=f"acc_ps{t}")
            nc.tensor.matmul(
                ps[t],
                lhsT=wttb[:, ts(t, P)],
                rhs=b_sb,
                start=True,
                stop=False,
            )

        # --- Main loop: experts in 4 pairs; per (pair, tile) one DVE mul
        # folds the routing weights into x^T, then 2x4 matmuls accumulate
        # into the tile's psum bank.
        for g in range(GROUPS):
            for t in range(N_TILES):
                xs = xspool.tile([P, CPG, KS, P], BF16)
                nc.vector.tensor_mul(
                    xs,
                    xt_sb[:, None, :, ts(t, P)].to_broadcast([P, CPG, KS, P]),
                    wb[:, ts(g, CPG), None, ts(t, P)].to_broadcast(
                        [P, CPG, KS, P]
                    ),
                )
                for ci in range(CPG):
                    c = g * CPG + ci
                    for k in range(KS):
                        last = g == GROUPS - 1 and ci == CPG - 1 and k == KS - 1
                        nc.tensor.matmul(
                            ps[t],
                            lhsT=xs[:, ci, k, :],
                            rhs=w_sb[:, c, k, :],
                            start=False,
                            stop=last,
                        )
                if g == GROUPS - 1:
                    o_sb = opool.tile([P, D_OUT], F32, tag="o_sb")
                    nc.scalar.copy(o_sb, ps[t])
                    nc.scalar.dma_start(o_d[ts(t, P), :], o_sb)


def _split_multi_waits(bir: dict) -> dict:
    """The walrus build in this container supports at most ONE sync-wait per
    instruction ("Too many sync wait commands" at codegen otherwise).  Tile's
    scheduler freely attaches several.  Split: keep the last wait on the
    instruction and hoist the others onto standalone same-engine
    EventSemaphore instructions inserted immediately before it — identical
    semantics (the engine blocks at the same program point)."""
    ctr = 0
    for func in bir["functions"]:
        for bb in func["blocks"]:
            new_insts = []
            for inst in bb["instructions"]:
                si = inst.get("sync_info")
                waits = si.get("on_wait") if si else None
                if waits and len(waits) > 1:
                    for w in waits[:-1]:
                        ctr += 1
                        new_insts.append(
                            {
                                "debug": inst.get("debug", 0),
                                "engine": inst["engine"],
                                "ins": [],
                                "outs": [],
                                "name": f"{inst['name']}-wsplit{ctr}",
                                "opcode": "EventSemaphore",
                                "sync_info": {"on_update": [], "on_wait": [w]},
                            }
                        )
                    si["on_wait"] = [waits[-1]]
                new_insts.append(inst)
            bb["instructions"] = new_insts
    return bir


_ORIG_TO_JSON_BYTES = bass.Bass.to_json_bytes


def _patched_to_json_bytes(self) -> bytes:
    bir = json.loads(_ORIG_TO_JSON_BYTES(self))
    _split_multi_waits(bir)
    return json.dumps(bir).encode()


_NC_CACHE = []


def _build() -> bass.Bass:
    if _NC_CACHE:
        return _NC_CACHE[0]
    nc = bass.Bass(
        "TRN2",
        target_bir_lowering=False,
        debug=False,
        enable_asserts=False,
        num_devices=N_CORES,
    )
    xt_d = nc.dram_tensor("xt_in", [D_IN, B_SHARD], BF16, kind="ExternalInput").ap()
    wb_d = nc.dram_tensor(
        "wb_in", [P, N_CTRL, B_SHARD], BF16, kind="ExternalInput"
    ).ap()
    wttb_d = nc.dram_tensor(
        "wttb_in", [N_CTRL, B_SHARD + D_OUT], BF16, kind="ExternalInput"
    ).ap()
    w_d = nc.dram_tensor("w_in", [N_CTRL, D_IN, D_OUT], BF16, kind="ExternalInput").ap()
    o_d = nc.dram_tensor("out", [B_SHARD, D_OUT], F32, kind="ExternalOutput").ap()
    with tile.TileContext(nc) as tc:
        _body(nc, tc, xt_d, wb_d, wttb_d, w_d, o_d)
    nc.to_json_bytes = types.MethodType(_patched_to_json_bytes, nc)
    _NC_CACHE.append(nc)
    return nc


def kernel(inputs, weights, w, b, _trace=False):
    nc = _build()
    inputs = np.ascontiguousarray(inputs, dtype=np.float32)
    weights = np.ascontiguousarray(weights, dtype=np.float32)

    w_bf = np.ascontiguousarray(w, dtype=np.float32).astype(ml_dtypes.bfloat16)
    b_bf = np.ascontiguousarray(b, dtype=np.float32).astype(ml_dtypes.bfloat16)

    in_maps = []
    for i in range(N_CORES):
        sl = slice(i * B_SHARD, (i + 1) * B_SHARD)
        x_sh = inputs[sl]                       # [B_SHARD, D_IN] f32
        wt_sh = weights[sl]                     # [B_SHARD, N_CTRL] f32
        xt = np.ascontiguousarray(x_sh.T).astype(ml_dtypes.bfloat16)
        wtt = np.ascontiguousarray(wt_sh.T).astype(ml_dtypes.bfloat16)
        wttb = np.concatenate([wtt, b_bf], axis=1)
        wb = np.broadcast_to(wtt[None, :, :], (P, N_CTRL, B_SHARD))
        wb = np.ascontiguousarray(wb)
        in_maps.append(
            {
                "xt_in": xt,
                "wb_in": wb,
                "wttb_in": wttb,
                "w_in": w_bf,
            }
        )
    res = run_bass_kernel_spmd(
        nc, in_maps, core_ids=list(range(N_CORES)), trace=_trace
    )
    out = np.concatenate([r["out"] for r in res.results], axis=0)
    if _trace:
        return out, res
    return out


# revision 10
# speedup vs baseline: 1.0623x; 1.0623x over previous
"""Mode-adaptive linear (MoE soft routing) Trainium2 kernel.

out[b, o] = sum_c weights[b, c] * (inputs[b, :] @ w[c])[o] + (weights @ bias)[b, o]

Strategy: data-parallel shard of the batch across 8 NeuronCores (1024 rows
each); w/bias replicated.  Host stages the inputs in matmul-ready form:
x pre-transposed to [D_IN, B_SHARD] bf16, w as bf16, and the routing weights
replicated across partitions (wb[p, c, b] = weights[b, c]).  On-device the
routing weights are folded into the x^T tiles with one DVE multiply per
(expert-pair, batch-tile), and all 8 expert matmuls plus the bias matmul
accumulate into a single PSUM bank per 128-row batch tile (8 tiles = 8 banks,
each a K=4104 accumulation).  Experts stream from HBM in 4 pairs so the PE
never waits on the bulk weight DMA.
"""

import json
import types

import ml_dtypes
import numpy as np

import concourse.bass as bass
import concourse.mybir as mybir
import concourse.tile as tile
from concourse.bass import ts
from concourse.bass_utils import run_bass_kernel_spmd

N_CORES = 8
B, D_IN, D_OUT, N_CTRL = 8192, 512, 512, 8
B_SHARD = B // N_CORES          # 1024 rows per core
P = 128
N_TILES = B_SHARD // P          # 8 batch tiles per core
KS = D_IN // P                  # 4 K-chunks of 128
GROUPS = 4                      # expert pairs
CPG = N_CTRL // GROUPS          # 2 experts per group
F32 = mybir.dt.float32
BF16 = mybir.dt.bfloat16
N_WARM = 6


def _body(nc: bass.Bass, tc: tile.TileContext, xt_d, wb_d, wttb_d, w_d, o_d):
    with (
        tc.tile_pool(name="const", bufs=1) as const_pool,
        tc.tile_pool(name="xspool", bufs=10) as xspool,
        tc.tile_pool(name="opool", bufs=3) as opool,
        tc.tile_pool(name="mm_ps", bufs=8, space="PSUM") as mm_psum,
    ):
        # --- Warmup: PE p-state / HAM ramps to full clock only after a few
        # us of continuous activity.  A handful of dummy matmuls on a memset
        # tile bridge the gap until the first real (bias) matmuls; the 8 bias
        # matmuls then continue the ramp with real work.  The warm psum tile
        # shares bank 0 with batch-tile 7's accumulator (PE is in-order, so
        # the warmups retire long before tile 7's group starts).
        warm_sb = const_pool.tile([P, P], BF16)
        nc.gpsimd.memset(warm_sb, 0.0)
        warm_ps = mm_psum.tile([P, 512], F32, tag="acc")
        for _ in range(N_WARM):
            nc.tensor.matmul(
                warm_ps[:, 0:P], lhsT=warm_sb, rhs=warm_sb,
                start=True, stop=True,
            )

        # --- Leading-wave DMAs (~2.2MB): just what the bias matmuls and the
        # first expert-pair's first tiles need.  DMA engines round-robin
        # packets across ALL active transfers (issue order is not priority),
        # so the bulk waves below are explicitly gated on compute progress to
        # keep them from stealing bandwidth from this leading wave.
        wttb = const_pool.tile([N_CTRL, B_SHARD + D_OUT], BF16)
        nc.scalar.dma_start(wttb, wttb_d)

        xt_sb = const_pool.tile([P, KS, B_SHARD], BF16)
        xt_src = xt_d.rearrange("(k p) b -> p k b", p=P)
        wb = const_pool.tile([P, N_CTRL, B_SHARD], BF16)
        nc.scalar.dma_start(xt_sb[:, :, 0:256], xt_src[:, :, 0:256])
        nc.scalar.dma_start(wb[:, 0:CPG, 0:256], wb_d[:, 0:CPG, 0:256])

        # gpsimd (software DGE, own queue): rest of x^T and first-pair wb.
        nc.gpsimd.dma_start(xt_sb[:, :, 256:B_SHARD], xt_src[:, :, 256:B_SHARD])
        nc.gpsimd.dma_start(
            wb[:, 0:CPG, 256:B_SHARD], wb_d[:, 0:CPG, 256:B_SHARD]
        )

        # sync: first expert-pair weights, finely chunked at the leading edge.
        w_sb = const_pool.tile([P, N_CTRL, KS, D_OUT], BF16)
        w_src = w_d.rearrange("c (k p) o -> p c k o", p=P)
        nc.sync.dma_start(w_sb[:, 0, 0], w_src[:, 0, 0])
        nc.sync.dma_start(w_sb[:, 0, 1:KS], w_src[:, 0, 1:KS])
        nc.sync.dma_start(w_sb[:, 1], w_src[:, 1])

        # --- Bias matmuls first: they only need the tiny wttb DMA, so they
        # start earliest, open all 8 psum accumulation groups, and double as
        # p-state ramp work.  lhsT = weights^T tile (K=8), rhs = bias.
        ps = [None] * N_TILES
        b_sb = wttb[:, B_SHARD:]
        for t in range(N_TILES):
            ps[t] = mm_psum.tile([P, D_OUT], F32, tag="acc", name=f"acc_ps{t}")
            nc.tensor.matmul(
                ps[t],
                lhsT=wttb[:, ts(t, P)],
                rhs=b_sb,
                start=True,
                stop=False,
            )

        # --- Main loop: experts in 4 pairs; per (pair, tile) one DVE mul
        # folds the routing weights into x^T, then 2x4 matmuls accumulate
        # into the tile's psum bank.  After xs(g, t=1) exists, release the
        # next bulk DMA wave: a 4-element gpsimd copy from the xs tile into
        # each wave destination makes the wave DMAs (WAW) wait for it, so
        # their transfers only start once the leading wave has drained.
        for g in range(GROUPS):
            for t in range(N_TILES):
                xs = xspool.tile([P, CPG, KS, P], BF16)
                nc.vector.tensor_mul(
                    xs,
                    xt_sb[:, None, :, ts(t, P)].to_broadcast([P, CPG, KS, P]),
                    wb[:, ts(g, CPG), None, ts(t, P)].to_broadcast(
                        [P, CPG, KS, P]
                    ),
                )
                if t == 1 and g == 0:
                    # Wave A: expert pair 1 (c2, c3).
                    nc.gpsimd.tensor_copy(w_sb[0:1, 2, 0, 0:4], xs[0:1, 0, 0, 0:4])
                    nc.gpsimd.tensor_copy(wb[0:1, 2, 0:4], xs[0:1, 0, 0, 4:8])
                    nc.sync.dma_start(w_sb[:, 2:4], w_src[:, 2:4])
                    nc.gpsimd.dma_start(wb[:, 2:4], wb_d[:, 2:4])
                if t == 1 and g == 1:
                    # Wave B: expert pairs 2 and 3 (c4..c7).
                    nc.gpsimd.tensor_copy(w_sb[0:1, 4, 0, 0:4], xs[0:1, 0, 0, 0:4])
                    nc.gpsimd.tensor_copy(wb[0:1, 4, 0:4], xs[0:1, 0, 0, 4:8])
                    nc.sync.dma_start(w_sb[:, 4:8], w_src[:, 4:8])
                    nc.gpsimd.dma_start(wb[:, 4:8], wb_d[:, 4:8])
                for ci in range(CPG):
                    c = g * CPG + ci
                    for k in range(KS):
                        last = g == GROUPS - 1 and ci == CPG - 1 and k == KS - 1
                        nc.tensor.matmul(
                            ps[t],
                            lhsT=xs[:, ci, k, :],
                            rhs=w_sb[:, c, k, :],
                            start=False,
                            stop=last,
                        )
                if g == GROUPS - 1:
                    o_sb = opool.tile([P, D_OUT], F32, tag="o_sb")
                    nc.scalar.copy(o_sb, ps[t])
                    nc.scalar.dma_start(o_d[ts(t, P), :], o_sb)


def _split_multi_waits(bir: dict) -> dict:
    """The walrus build in this container supports at most ONE sync-wait per
    instruction ("Too many sync wait commands" at codegen otherwise).  Tile's
    scheduler freely attaches several.  Split: keep the last wait on the
    instruction and hoist the others onto standalone same-engine
    EventSemaphore instructions inserted immediately before it — identical
    semantics (the engine blocks at the same program point)."""
    ctr = 0
    for func in bir["functions"]:
        for bb in func["blocks"]:
            new_insts = []
            for inst in bb["instructions"]:
                si = inst.get("sync_info")
                waits = si.get("on_wait") if si else None
                if waits and len(waits) > 1:
                    for w in waits[:-1]:
                        ctr += 1
                        new_insts.append(
                            {
                                "debug": inst.get("debug", 0),
                                "engine": inst["engine"],
                                "ins": [],
                                "outs": [],
                                "name": f"{inst['name']}-wsplit{ctr}",
                                "opcode": "EventSemaphore",
                                "sync_info": {"on_update": [], "on_wait": [w]},
                            }
                        )
                    si["on_wait"] = [waits[-1]]
                new_insts.append(inst)
            bb["instructions"] = new_insts
    return bir


_ORIG_TO_JSON_BYTES = bass.Bass.to_json_bytes


def _patched_to_json_bytes(self) -> bytes:
    bir = json.loads(_ORIG_TO_JSON_BYTES(self))
    _split_multi_waits(bir)
    return json.dumps(bir).encode()


_NC_CACHE = []


def _build() -> bass.Bass:
    if _NC_CACHE:
        return _NC_CACHE[0]
    nc = bass.Bass(
        "TRN2",
        target_bir_lowering=False,
        debug=False,
        enable_asserts=False,
        num_devices=N_CORES,
    )
    xt_d = nc.dram_tensor("xt_in", [D_IN, B_SHARD], BF16, kind="ExternalInput").ap()
    wb_d = nc.dram_tensor(
        "wb_in", [P, N_CTRL, B_SHARD], BF16, kind="ExternalInput"
    ).ap()
    wttb_d = nc.dram_tensor(
        "wttb_in", [N_CTRL, B_SHARD + D_OUT], BF16, kind="ExternalInput"
    ).ap()
    w_d = nc.dram_tensor("w_in", [N_CTRL, D_IN, D_OUT], BF16, kind="ExternalInput").ap()
    o_d = nc.dram_tensor("out", [B_SHARD, D_OUT], F32, kind="ExternalOutput").ap()
    with tile.TileContext(nc) as tc:
        _body(nc, tc, xt_d, wb_d, wttb_d, w_d, o_d)
    nc.to_json_bytes = types.MethodType(_patched_to_json_bytes, nc)
    _NC_CACHE.append(nc)
    return nc


def kernel(inputs, weights, w, b, _trace=False):
    nc = _build()
    inputs = np.ascontiguousarray(inputs, dtype=np.float32)
    weights = np.ascontiguousarray(weights, dtype=np.float32)

    w_bf = np.ascontiguousarray(w, dtype=np.float32).astype(ml_dtypes.bfloat16)
    b_bf = np.ascontiguousarray(b, dtype=np.float32).astype(ml_dtypes.bfloat16)

    in_maps = []
    for i in range(N_CORES):
        sl = slice(i * B_SHARD, (i + 1) * B_SHARD)
        x_sh = inputs[sl]                       # [B_SHARD, D_IN] f32
        wt_sh = weights[sl]                     # [B_SHARD, N_CTRL] f32
        xt = np.ascontiguousarray(x_sh.T).astype(ml_dtypes.bfloat16)
        wtt = np.ascontiguousarray(wt_sh.T).astype(ml_dtypes.bfloat16)
        wttb = np.concatenate([wtt, b_bf], axis=1)
        wb = np.broadcast_to(wtt[None, :, :], (P, N_CTRL, B_SHARD))
        wb = np.ascontiguousarray(wb)
        in_maps.append(
            {
                "xt_in": xt,
                "wb_in": wb,
                "wttb_in": wttb,
                "w_in": w_bf,
            }
        )
    res = run_bass_kernel_spmd(
        nc, in_maps, core_ids=list(range(N_CORES)), trace=_trace
    )
    out = np.concatenate([r["out"] for r in res.results], axis=0)
    if _trace:
        return out, res
    return out
